# revision 17
# baseline (speedup 1.0000x reference)
"""Multi-head attention (B=4, S=2048, D=1024, H=16) on 8 TRN2 NeuronCores.

Sharding: core c handles HEAD-PAIR c (heads 2c, 2c+1) of EVERY batch.
Per-batch attention cost scales with n_kt(b) = ceil(valid_len/128) key
tiles, so each core's total work is sum_b n_kt(b) k-tiles — identical
across cores for any valid_lens (perfect balance), vs 8 heads x
max_b n_kt for a batch-sharded layout. Row-parallel Wo gives per-core
partial outputs per batch; the host sums the 8 partials (the unshard
step of row-parallel layout).

Per-core dataflow per batch-slot v (batches sorted by n_kt desc):
  KT[d',k] = (Xk Wk_pair)^T over valid k only   (bf16, d'=128 = 2 heads)
  QT[d',q] = (Xq Wq_pair)^T                      (bf16)
  V[k,2,65] = Xv Wv_pair, col 64 = ones          (bf16)
  per 512-q chunk, per k-tile:
    scores[k, 2x512]: 2 concurrent MMs (heads at partitions 0:64/64:128)
    e = exp(scores*scale + maskbias)             (one ACT instr, N=1024)
    av[65, 2x512]   += V^T-ish @ e               (row 64 = softmax denom)
  rr = reciprocal_approx_fast(av[64])            (DVE, f32, from PSUM)
  OT = av[0:64] * bc(rr)                         (bc via K=1 f32r matmul)
  out_v[q,:] = OT^T-slice.T @ Wo_pair            (partial; host sums)

A background PE-task FIFO interleaves later batches' projections and
finished chunks' output projections between attention k-tile iterations
so the PE stream stays dense (HAM stays warm) while ACT crunches exps.
PSUM budget: scores 2 banks + av 2x2 banks + 2 rotating bg banks.

Masking: key positions k >= valid_len get exp() forced to 0 via
bias=-1e6 (exp(-1e6) underflows to exactly 0.0 in f32, matching the
reference softmax given its -1e6 fill). valid_len==0 gives uniform
attention (scale=0, bias=0 -> exp(0)=1) over all 16 k-tiles, matching
jax.nn.softmax on an all-masked row.
"""

import math

import numpy as np

B, S, D, H = 4, 2048, 1024, 16
HD = D // H  # 64
NCORES = 8
PC = 128  # head-pair dims per core (2 heads x 64)
NEG = -1.0e6
P = 128

_PROG_CACHE = {}


def _patch_tile_drain():
    """The walrus build in this container rejects sem waits attached to the
    Tile end-of-kernel Drain ("Too many sync wait commands" / SIGABRT).
    Replace them with standalone EventSemaphore waits, which it accepts."""
    import concourse.tile as tile
    from concourse.vector_clock import ScopedClock

    if getattr(tile.TileContext, "_drain_patched", False):
        return

    def _drain_and_barrier(self, tick_clock, wait_clock):
        nc = self.nc
        drain_inst = nc.sync.drain()
        wait_clock.add_sem_waits(
            drain_inst.ins, ScopedClock({None: tick_clock.global_clock})
        )
        si = drain_inst.ins.sync_info
        waits = list(si.on_wait) if si is not None and si.on_wait else []
        if waits:
            si.on_wait.clear()
            by_id, by_name = {}, {}
            for h in wait_clock.sems.allocated().values():
                by_id[getattr(h, "id", None)] = h
                by_name[getattr(h, "name", None)] = h
            for w in waits:
                h = by_id.get(w.id) or by_name.get(w.ant_name)
                assert h is not None, f"no handle for sem {w.ant_name} ({w.id})"
                nc.sync.wait_ge(h, w.wait_value)
        nc.all_engine_barrier()
        assert self.sems is not None
        popped = nc._tile_sem_poison_stack.pop()
        assert popped is self._sem_poison
        nc.clear_and_free_semaphores(list(self.sems.allocated().values()))
        nc.all_engine_barrier()

    tile.TileContext._drain_and_barrier = _drain_and_barrier
    tile.TileContext._drain_patched = True


def _split_multi_waits(nc, mybir):
    """This container's walrus rejects instructions carrying more than one
    semaphore wait ("Too many sync wait commands"). Hoist excess waits into
    standalone EventSemaphore instructions on the same engine, inserted
    immediately before the instruction — same-engine stream order preserves
    the semantics exactly."""
    n_ev = 0
    for fn in nc.m.functions:
        for bb in fn.blocks:
            insts = bb.instructions
            out = []
            for inst in insts:
                si = inst.sync_info
                waits = list(si.on_wait) if si is not None and si.on_wait else []
                keep = 0 if inst.opcode == "Drain" else 1
                if len(waits) > keep:
                    excess = waits[: len(waits) - keep]
                    kept = waits[len(waits) - keep:]
                    si.on_wait.clear()
                    si.on_wait.extend(kept)
                    for w in excess:
                        ev = mybir.InstEventSemaphore(
                            name=f"{inst.name}-hw{n_ev}",
                            engine=inst.engine,
                        )
                        ev.sync_info = mybir.SyncInfo(on_wait=[w], on_update=[])
                        out.append(ev)
                        n_ev += 1
                out.append(inst)
            if n_ev:
                insts[:] = out
    return n_ev


def _build_program(kts: tuple):
    import concourse.bass as bass
    import concourse.mybir as mybir
    import concourse.tile as tile

    _patch_tile_drain()

    f32 = mybir.dt.float32
    f32r = mybir.dt.float32r
    bf16 = mybir.dt.bfloat16
    AF = mybir.ActivationFunctionType

    NB = len(kts)
    SL = sum(kts)  # total k tiles across batch slots
    off = [0] * NB
    for v in range(1, NB):
        off[v] = off[v - 1] + kts[v - 1]

    nc = bass.Bass()

    xq_d = [nc.dram_tensor(f"xq{v}", [D, S], bf16, kind="ExternalInput")
            for v in range(NB)]
    xk_d = [nc.dram_tensor(f"xk{v}", [D, kts[v] * P], bf16, kind="ExternalInput")
            for v in range(NB)]
    xv_d = [nc.dram_tensor(f"xv{v}", [D, kts[v] * P], bf16, kind="ExternalInput")
            for v in range(NB)]
    wq_d = nc.dram_tensor("wq", [D, PC], bf16, kind="ExternalInput")
    wk_d = nc.dram_tensor("wk", [D, PC], bf16, kind="ExternalInput")
    wv_d = nc.dram_tensor("wv", [D, PC], bf16, kind="ExternalInput")
    wo_d = nc.dram_tensor("wo", [PC, D], bf16, kind="ExternalInput")
    mb_d = nc.dram_tensor("mb", [P, SL], f32, kind="ExternalInput")
    ms_d = nc.dram_tensor("ms", [P, SL], f32, kind="ExternalInput")
    out_d = [nc.dram_tensor(f"out{v}", [S, D], bf16, kind="ExternalOutput")
             for v in range(NB)]

    with tile.TileContext(nc) as tc:
        with (
            tc.tile_pool(name="pp", bufs=1) as pp,
            tc.tile_pool(name="xtp", bufs=8) as xtp,
            tc.tile_pool(name="expp", bufs=3) as expp,
            tc.tile_pool(name="nrm", bufs=3) as nrm,
            tc.tile_pool(name="outp", bufs=3) as outp,
            tc.tile_pool(name="psA", bufs=2, space="PSUM") as psA,
            tc.tile_pool(name="psB", bufs=1, space="PSUM") as psB,
            tc.tile_pool(name="psC", bufs=2, space="PSUM") as psC,
        ):
            # ---- persistent tensors
            KT = pp.tile([P, SL * P], bf16, name="KT")
            QT = pp.tile([P, NB, S], bf16, name="QT")
            V = pp.tile([P, SL, 2, HD + 1], bf16, name="V")
            OT = pp.tile([P, NB, S], bf16, name="OT")
            ones1 = pp.tile([1, P], bf16, name="ones1")
            mb = pp.tile([P, SL], f32, name="mb")
            msc = pp.tile([P, SL], f32, name="msc")
            wq = pp.tile([P, 8, PC], bf16, name="wq")
            wk = pp.tile([P, 8, PC], bf16, name="wk")
            wv = pp.tile([P, 8, PC], bf16, name="wv")
            wo = pp.tile([PC, D], bf16, name="wo")

            nc.any.memset(ones1[:], 1.0)
            nc.any.memset(V[:, :, :, HD:HD + 1], 1.0)
            nc.sync.dma_start(mb[:], mb_d[:, :])
            nc.sync.dma_start(msc[:], ms_d[:, :])
            nc.sync.dma_start(wq[:], wq_d[:, :].rearrange("(a p) c -> p a c", p=P))
            nc.sync.dma_start(wk[:], wk_d[:, :].rearrange("(a p) c -> p a c", p=P))
            nc.sync.dma_start(wv[:], wv_d[:, :].rearrange("(a p) c -> p a c", p=P))
            nc.sync.dma_start(wo[:], wo_d[:, :])

            # ---- background PE task FIFO.
            # Each item is (dma_fn | None, mm_fn).  dma_fn is emitted
            # LOOKAHEAD items early so transfers are in flight before the
            # matmuls need them; mm_fn is emitted between attention k-tile
            # iterations to keep the PE stream dense without making the
            # attention matmuls transitively wait on distant DMAs.
            bgq = []
            bg_next_mm = [0]
            bg_next_dma = [0]
            LOOKAHEAD = 7

            def bg_prefetch():
                while (
                    bg_next_dma[0] < len(bgq)
                    and bg_next_dma[0] < bg_next_mm[0] + LOOKAHEAD
                ):
                    dma_fn, _ = bgq[bg_next_dma[0]]
                    if dma_fn is not None:
                        dma_fn()
                    bg_next_dma[0] += 1

            def bg_pop(n=1):
                bg_prefetch()
                for _ in range(n):
                    if bg_next_mm[0] >= len(bgq):
                        return
                    _, mm_fn = bgq[bg_next_mm[0]]
                    bg_next_mm[0] += 1
                    mm_fn()
                    bg_prefetch()

            def bg_drain():
                bg_pop(len(bgq) - bg_next_mm[0] + 1)

            # ---- projection tasks (enqueued, popped as bg work)
            def kt_task(v, sl):
                SK = kts[v] * P
                w = min(512, SK - sl * 512)
                x_re = xk_d[v][:, :].rearrange("(a p) k -> p a k", p=P)

                def dma_fn():
                    xs = xtp.tile([P, 8, 512], bf16, name="xs", tag="xt")
                    nc.sync.dma_start(
                        xs[:, :, :w], x_re[:, :, sl * 512:sl * 512 + w]
                    )
                    dma_fn.xs = xs

                def mm_fn():
                    xs = dma_fn.xs
                    pj = psC.tile([P, 512], f32, name="pj", tag="bg")
                    for a in range(8):
                        nc.tensor.matmul(
                            pj[:, :w],
                            lhsT=wk[:, a, :],
                            rhs=xs[:, a, :w],
                            start=(a == 0),
                            stop=(a == 7),
                        )
                    base = off[v] * P + sl * 512
                    nc.vector.tensor_copy(out=KT[:, base:base + w], in_=pj[:, :w])

                return (dma_fn, mm_fn)

            def v_task(v, sl):
                SK = kts[v] * P
                w = min(512, SK - sl * 512)
                nsub = w // P
                x_re = xv_d[v][:, :].rearrange("(a p) k -> p a k", p=P)

                def dma_fn():
                    xs = xtp.tile([P, 8, 512], bf16, name="xvs", tag="xt")
                    nc.sync.dma_start(
                        xs[:, :, :w], x_re[:, :, sl * 512:sl * 512 + w]
                    )
                    dma_fn.xs = xs

                def mm_fn():
                    xs = dma_fn.xs
                    for sub in range(nsub):
                        kt = off[v] + sl * 4 + sub
                        pv = psC.tile([P, 512], f32, name="pv", tag="bg")
                        for a in range(8):
                            nc.tensor.matmul(
                                pv[:, 0:PC],
                                lhsT=xs[:, a, sub * P:(sub + 1) * P],
                                rhs=wv[:, a, :],
                                start=(a == 0),
                                stop=(a == 7),
                            )
                        nc.vector.tensor_copy(
                            out=V[:, kt, :, 0:HD],
                            in_=pv[:, 0:PC].rearrange("p (h c) -> p h c", c=HD),
                        )

                return (dma_fn, mm_fn)

            def qt_task(v, sl):
                x_re = xq_d[v][:, :].rearrange("(a p) q -> p a q", p=P)

                def dma_fn():
                    xs = xtp.tile([P, 8, 512], bf16, name="xqs", tag="xt")
                    nc.sync.dma_start(xs[:], x_re[:, :, sl * 512:(sl + 1) * 512])
                    dma_fn.xs = xs

                def mm_fn():
                    xs = dma_fn.xs
                    pq = psC.tile([P, 512], f32, name="pq", tag="bg")
                    for a in range(8):
                        nc.tensor.matmul(
                            pq[:],
                            lhsT=wq[:, a, :],
                            rhs=xs[:, a, :],
                            start=(a == 0),
                            stop=(a == 7),
                        )
                    nc.vector.tensor_copy(
                        out=QT[:, v, sl * 512:(sl + 1) * 512], in_=pq[:]
                    )

                return (dma_fn, mm_fn)

            def d_task(v, qt):
                # output projection: out_v[qt*128:, :] = OT_v_slice.T @ Wo
                def mm_fn():
                    ob = outp.tile([P, D], bf16, name="ob", tag="ob")
                    for eh in range(2):
                        wd = psC.tile([P, 512], f32, name="wd", tag="bg")
                        nc.tensor.matmul(
                            wd[:],
                            lhsT=OT[:, v, qt * P:(qt + 1) * P],
                            rhs=wo[:, eh * 512:(eh + 1) * 512],
                            start=True,
                            stop=True,
                        )
                        nc.vector.tensor_copy(
                            out=ob[:, eh * 512:(eh + 1) * 512], in_=wd[:]
                        )
                    nc.sync.dma_start(out_d[v][qt * P:(qt + 1) * P, :], ob[:])

                return (None, mm_fn)

            def nslabs(v):
                return (kts[v] * P + 511) // 512

            # D tasks deferred by one chunk so their normalization chain
            # (copy->repack->recip->unpack->bc->mul->shift-DMA) has a chunk's
            # worth of time before a bg pop makes the in-order PE stream
            # wait on it.
            defer_d = []

            # ---- attention for one batch slot
            def attention(v, pops):
                L = kts[v]
                ko = off[v]
                for ch in range(4):  # 512-wide q chunks
                    bgq.extend(defer_d)
                    defer_d.clear()
                    q0 = ch * 512
                    av = psB.tile([HD + 1, 1024], f32, name="av", tag="av")
                    for kt in range(L):
                        bg_pop(pops)
                        sc = psA.tile([P, 1024], f32, name="sc", tag="sc")
                        for s in range(2):
                            pb = s * HD
                            nc.tensor.matmul(
                                sc[:, s * 512:(s + 1) * 512],
                                lhsT=KT[
                                    pb:pb + HD,
                                    (ko + kt) * P:(ko + kt + 1) * P,
                                ],
                                rhs=QT[pb:pb + HD, v, q0:q0 + 512],
                                start=True,
                                stop=True,
                            )
                        ex = expp.tile([P, 1024], bf16, name="ex", tag="ex")
                        nc.scalar.activation(
                            ex[:],
                            sc[:],
                            AF.Exp,
                            bias=mb[:, ko + kt:ko + kt + 1],
                            scale=msc[:, ko + kt:ko + kt + 1],
                        )
                        for s in range(2):
                            nc.tensor.matmul(
                                av[:, s * 512:(s + 1) * 512],
                                lhsT=V[:, ko + kt, s, :],
                                rhs=ex[:, s * 512:(s + 1) * 512],
                                start=(kt == 0),
                                stop=(kt == L - 1),
                            )
                    # normalization.  A [1,1024] reciprocal on the DVE costs
                    # ~6.4ns per free-dim column, so repack the denominator
                    # row to [8,128] (via a DRAM bounce — DMA is the only
                    # engine that can move data across partitions) and run
                    # the reciprocal over 128 columns instead of 1024.
                    avb = nrm.tile([HD + 1, 1024], bf16, name="avb", tag="rcp")
                    nc.vector.tensor_copy(out=avb[:], in_=av[:])
                    dct = nrm.tile([8, P], bf16, name="dct", tag="dc")
                    nc.sync.dma_start(
                        dct[0:8, :],
                        avb[HD:HD + 1, :].rearrange("o (p c) -> o p c", p=8),
                    )
                    with nc.allow_low_precision(
                        reason="softmax denominators are O(1e3); bf16 "
                        "reciprocal keeps enough digits for attention"
                    ):
                        nc.vector.reciprocal(dct[0:8, :], dct[0:8, :])
                    rrb = nrm.tile([1, 1024], bf16, name="rrb", tag="rrb")
                    nc.sync.dma_start(
                        rrb[:].rearrange("o (p c) -> o p c", p=8),
                        dct[0:8, :],
                    )
                    for s in range(2):
                        bc = psC.tile([P, 512], f32, name="bc", tag="bg")
                        nc.tensor.matmul(
                            bc[:],
                            lhsT=ones1[:, :],
                            rhs=rrb[:, s * 512:(s + 1) * 512],
                            start=True,
                            stop=True,
                        )
                        if s == 0:
                            nc.vector.tensor_mul(
                                out=OT[0:HD, v, q0:q0 + 512],
                                in0=avb[0:HD, 0:512],
                                in1=bc[0:HD, :],
                            )
                        else:
                            # DVE lanes can't shift partitions; write the odd
                            # head at partitions 0:64 and DMA it to 64:128
                            ot1 = nrm.tile([HD, 512], bf16, name="ot1", tag="ot1")
                            nc.vector.tensor_mul(
                                out=ot1[:],
                                in0=avb[0:HD, 512:1024],
                                in1=bc[0:HD, :],
                            )
                            nc.sync.dma_start(
                                OT[HD:P, v, q0:q0 + 512], ot1[:]
                            )
                    for qt in range(ch * 4, ch * 4 + 4):
                        defer_d.append(d_task(v, qt))

            # ---- emission schedule.
            # Only slab 0 of KT/V/QT(v0) is emitted inline (the PE stream is
            # in-order: anything emitted before the first scores MM would
            # make it transitively wait on that DMA).  Everything else goes
            # through the FIFO, popped one group per attention k-tile.
            kt0, v0, qt0 = kt_task(0, 0), v_task(0, 0), qt_task(0, 0)
            for dma_fn, _ in (kt0, v0, qt0):
                dma_fn()
            for _, mm_fn in (kt0, v0, qt0):
                mm_fn()
            for sl in range(1, nslabs(0)):
                bgq.append(kt_task(0, sl))
                bgq.append(v_task(0, sl))
            for sl in range(1, 4):
                bgq.append(qt_task(0, sl))
            for vv in range(1, NB):
                for sl in range(nslabs(vv)):
                    bgq.append(kt_task(vv, sl))
                    bgq.append(v_task(vv, sl))
                for sl in range(4):
                    bgq.append(qt_task(vv, sl))

            for v in range(NB):
                attention(v, pops=2 if v < NB - 1 else 1)
            bgq.extend(defer_d)
            defer_d.clear()
            bg_drain()

    _split_multi_waits(nc, mybir)
    return nc


def _get_program(kts: tuple):
    if kts not in _PROG_CACHE:
        _PROG_CACHE[kts] = _build_program(kts)
    return _PROG_CACHE[kts]


def kernel(**inputs) -> np.ndarray:
    import ml_dtypes
    from concourse.bass_utils import run_bass_kernel_spmd

    q = np.asarray(inputs["queries"], dtype=np.float32)
    k = np.asarray(inputs["keys"], dtype=np.float32)
    v = np.asarray(inputs["values"], dtype=np.float32)
    vl = np.asarray(inputs["valid_lens"]).astype(np.int64)
    Wq = np.asarray(inputs["Wq"], dtype=np.float32)
    Wk = np.asarray(inputs["Wk"], dtype=np.float32)
    Wv = np.asarray(inputs["Wv"], dtype=np.float32)
    Wo = np.asarray(inputs["Wo"], dtype=np.float32)

    bf = ml_dtypes.bfloat16
    nkt = np.array(
        [S // P if vl[b] == 0 else min(S // P, int(math.ceil(vl[b] / P)))
         for b in range(B)],
        dtype=np.int64,
    )
    # batches sorted by n_kt ASCENDING: the lead-in only waits on the
    # smallest batch's projections, and the longest batch's attention span
    # hides all earlier normalization chains and output projections — only
    # the very last chunk's chain is exposed as tail.
    order = np.argsort(nkt, kind="stable")
    kts = tuple(int(nkt[b]) for b in order)
    nc = _get_program(kts)

    # masks are identical across cores: [128, SL] per-(k-partition, k-tile)
    mbs, mss = [], []
    for vv, b in enumerate(order):
        L = kts[vv]
        kk = (np.arange(L)[None, :] * P + np.arange(P)[:, None]).astype(np.int64)
        vlb = int(vl[b])
        if vlb == 0:
            mbs.append(np.zeros((P, L), np.float32))
            mss.append(np.zeros((P, L), np.float32))
        else:
            mbs.append(np.where(kk < vlb, 0.0, NEG).astype(np.float32))
            mss.append(np.full((P, L), 1.0 / math.sqrt(HD), np.float32))
    m_bias = np.concatenate(mbs, axis=1)
    m_scale = np.concatenate(mss, axis=1)

    xqs = [np.ascontiguousarray(q[b].T).astype(bf) for b in order]
    xks = [np.ascontiguousarray(k[b].T[:, :kts[vv] * P]).astype(bf)
           for vv, b in enumerate(order)]
    xvs = [np.ascontiguousarray(v[b].T[:, :kts[vv] * P]).astype(bf)
           for vv, b in enumerate(order)]

    in_maps = []
    for c in range(NCORES):
        cols = slice(c * PC, (c + 1) * PC)
        im = {
            "wq": np.ascontiguousarray(Wq[:, cols]).astype(bf),
            "wk": np.ascontiguousarray(Wk[:, cols]).astype(bf),
            "wv": np.ascontiguousarray(Wv[:, cols]).astype(bf),
            "wo": np.ascontiguousarray(Wo[cols, :]).astype(bf),
            "mb": m_bias,
            "ms": m_scale,
        }
        for vv in range(len(kts)):
            im[f"xq{vv}"] = xqs[vv]
            im[f"xk{vv}"] = xks[vv]
            im[f"xv{vv}"] = xvs[vv]
        in_maps.append(im)

    globals()["_LAST_IN_MAPS"] = in_maps
    res = run_bass_kernel_spmd(nc, in_maps, list(range(NCORES))).results

    out = np.zeros((B, S, D), dtype=np.float32)
    for vv, b in enumerate(order):
        for c in range(NCORES):
            out[b] += res[c][f"out{vv}"].astype(np.float32)
    return out


# revision 19
# speedup vs baseline: 1.1819x; 1.1819x over previous
"""Multi-head attention (B=4, S=2048, D=1024, H=16) on 8 TRN2 NeuronCores.

Sharding: core c handles HEAD-PAIR c (heads 2c, 2c+1) of EVERY batch.
Per-batch attention cost scales with n_kt(b) = ceil(valid_len/128) key
tiles, so each core's total work is sum_b n_kt(b) k-tiles — identical
across cores for any valid_lens (perfect balance), vs 8 heads x
max_b n_kt for a batch-sharded layout. Row-parallel Wo gives per-core
partial outputs per batch; the host sums the 8 partials (the unshard
step of row-parallel layout).

Per-core dataflow per batch-slot v (batches sorted by n_kt desc):
  KT[d',k] = (Xk Wk_pair)^T over valid k only   (bf16, d'=128 = 2 heads)
  QT[d',q] = (Xq Wq_pair)^T                      (bf16)
  V[k,2,65] = Xv Wv_pair, col 64 = ones          (bf16)
  per 512-q chunk, per k-tile:
    scores[k, 2x512]: 2 concurrent MMs (heads at partitions 0:64/64:128)
    e = exp(scores*scale + maskbias)             (one ACT instr, N=1024)
    av[65, 2x512]   += V^T-ish @ e               (row 64 = softmax denom)
  rr = reciprocal_approx_fast(av[64])            (DVE, f32, from PSUM)
  OT = av[0:64] * bc(rr)                         (bc via K=1 f32r matmul)
  out_v[q,:] = OT^T-slice.T @ Wo_pair            (partial; host sums)

A background PE-task FIFO interleaves later batches' projections and
finished chunks' output projections between attention k-tile iterations
so the PE stream stays dense (HAM stays warm) while ACT crunches exps.
PSUM budget: scores 2 banks + av 2x2 banks + 2 rotating bg banks.

Masking: key positions k >= valid_len get exp() forced to 0 via
bias=-1e6 (exp(-1e6) underflows to exactly 0.0 in f32, matching the
reference softmax given its -1e6 fill). valid_len==0 gives uniform
attention (scale=0, bias=0 -> exp(0)=1) over all 16 k-tiles, matching
jax.nn.softmax on an all-masked row.
"""

import math

import numpy as np

B, S, D, H = 4, 2048, 1024, 16
HD = D // H  # 64
NCORES = 8
PC = 128  # head-pair dims per core (2 heads x 64)
NEG = -1.0e6
P = 128

_PROG_CACHE = {}


def _patch_tile_drain():
    """The walrus build in this container rejects sem waits attached to the
    Tile end-of-kernel Drain ("Too many sync wait commands" / SIGABRT).
    Replace them with standalone EventSemaphore waits, which it accepts."""
    import concourse.tile as tile
    from concourse.vector_clock import ScopedClock

    if getattr(tile.TileContext, "_drain_patched", False):
        return

    def _drain_and_barrier(self, tick_clock, wait_clock):
        nc = self.nc
        drain_inst = nc.sync.drain()
        wait_clock.add_sem_waits(
            drain_inst.ins, ScopedClock({None: tick_clock.global_clock})
        )
        si = drain_inst.ins.sync_info
        waits = list(si.on_wait) if si is not None and si.on_wait else []
        if waits:
            si.on_wait.clear()
            by_id, by_name = {}, {}
            for h in wait_clock.sems.allocated().values():
                by_id[getattr(h, "id", None)] = h
                by_name[getattr(h, "name", None)] = h
            for w in waits:
                h = by_id.get(w.id) or by_name.get(w.ant_name)
                assert h is not None, f"no handle for sem {w.ant_name} ({w.id})"
                nc.sync.wait_ge(h, w.wait_value)
        nc.all_engine_barrier()
        assert self.sems is not None
        popped = nc._tile_sem_poison_stack.pop()
        assert popped is self._sem_poison
        nc.clear_and_free_semaphores(list(self.sems.allocated().values()))
        nc.all_engine_barrier()

    tile.TileContext._drain_and_barrier = _drain_and_barrier
    tile.TileContext._drain_patched = True


def _split_multi_waits(nc, mybir):
    """This container's walrus rejects instructions carrying more than one
    semaphore wait ("Too many sync wait commands"). Hoist excess waits into
    standalone EventSemaphore instructions on the same engine, inserted
    immediately before the instruction — same-engine stream order preserves
    the semantics exactly."""
    n_ev = 0
    for fn in nc.m.functions:
        for bb in fn.blocks:
            insts = bb.instructions
            out = []
            for inst in insts:
                si = inst.sync_info
                waits = list(si.on_wait) if si is not None and si.on_wait else []
                keep = 0 if inst.opcode == "Drain" else 1
                if len(waits) > keep:
                    excess = waits[: len(waits) - keep]
                    kept = waits[len(waits) - keep:]
                    si.on_wait.clear()
                    si.on_wait.extend(kept)
                    for w in excess:
                        ev = mybir.InstEventSemaphore(
                            name=f"{inst.name}-hw{n_ev}",
                            engine=inst.engine,
                        )
                        ev.sync_info = mybir.SyncInfo(on_wait=[w], on_update=[])
                        out.append(ev)
                        n_ev += 1
                out.append(inst)
            if n_ev:
                insts[:] = out
    return n_ev


def _build_program(kts: tuple):
    import concourse.bass as bass
    import concourse.mybir as mybir
    import concourse.tile as tile

    _patch_tile_drain()

    f32 = mybir.dt.float32
    f32r = mybir.dt.float32r
    bf16 = mybir.dt.bfloat16
    AF = mybir.ActivationFunctionType

    NB = len(kts)
    SL = sum(kts)  # total k tiles across batch slots
    off = [0] * NB
    for v in range(1, NB):
        off[v] = off[v - 1] + kts[v - 1]

    nc = bass.Bass()

    xq_d = [nc.dram_tensor(f"xq{v}", [D, S], bf16, kind="ExternalInput")
            for v in range(NB)]
    xk_d = [nc.dram_tensor(f"xk{v}", [D, kts[v] * P], bf16, kind="ExternalInput")
            for v in range(NB)]
    xv_d = [nc.dram_tensor(f"xv{v}", [D, kts[v] * P], bf16, kind="ExternalInput")
            for v in range(NB)]
    wq_d = nc.dram_tensor("wq", [D, PC], bf16, kind="ExternalInput")
    wk_d = nc.dram_tensor("wk", [D, PC], bf16, kind="ExternalInput")
    wv_d = nc.dram_tensor("wv", [D, PC], bf16, kind="ExternalInput")
    wo_d = nc.dram_tensor("wo", [PC, D], bf16, kind="ExternalInput")
    mb_d = nc.dram_tensor("mb", [P, SL], f32, kind="ExternalInput")
    ms_d = nc.dram_tensor("ms", [P, SL], f32, kind="ExternalInput")
    out_d = [nc.dram_tensor(f"out{v}", [S, D], bf16, kind="ExternalOutput")
             for v in range(NB)]

    with tile.TileContext(nc) as tc:
        with (
            tc.tile_pool(name="pp", bufs=1) as pp,
            tc.tile_pool(name="xtp", bufs=8) as xtp,
            tc.tile_pool(name="expp", bufs=3) as expp,
            tc.tile_pool(name="nrm", bufs=3) as nrm,
            tc.tile_pool(name="outp", bufs=3) as outp,
            tc.tile_pool(name="psA", bufs=2, space="PSUM") as psA,
            tc.tile_pool(name="psB", bufs=1, space="PSUM") as psB,
            tc.tile_pool(name="psC", bufs=2, space="PSUM") as psC,
        ):
            # ---- persistent tensors
            KT = pp.tile([P, SL * P], bf16, name="KT")
            QT = pp.tile([P, NB, S], bf16, name="QT")
            V = pp.tile([P, SL, 2, HD + 1], bf16, name="V")
            OT = pp.tile([P, NB, S], bf16, name="OT")
            ones1 = pp.tile([1, P], bf16, name="ones1")
            mb = pp.tile([P, SL], f32, name="mb")
            msc = pp.tile([P, SL], f32, name="msc")
            wq = pp.tile([P, 8, PC], bf16, name="wq")
            wk = pp.tile([P, 8, PC], bf16, name="wk")
            wv = pp.tile([P, 8, PC], bf16, name="wv")
            wo = pp.tile([PC, D], bf16, name="wo")

            nc.any.memset(ones1[:], 1.0)
            nc.any.memset(V[:, :, :, HD:HD + 1], 1.0)
            nc.sync.dma_start(mb[:], mb_d[:, :])
            nc.sync.dma_start(msc[:], ms_d[:, :])
            nc.sync.dma_start(wq[:], wq_d[:, :].rearrange("(a p) c -> p a c", p=P))
            nc.sync.dma_start(wk[:], wk_d[:, :].rearrange("(a p) c -> p a c", p=P))
            nc.sync.dma_start(wv[:], wv_d[:, :].rearrange("(a p) c -> p a c", p=P))
            nc.sync.dma_start(wo[:], wo_d[:, :])

            # ---- background PE task FIFO.
            # Each item is (dma_fn | None, mm_fn).  dma_fn is emitted
            # LOOKAHEAD items early so transfers are in flight before the
            # matmuls need them; mm_fn is emitted between attention k-tile
            # iterations to keep the PE stream dense without making the
            # attention matmuls transitively wait on distant DMAs.
            bgq = []
            bg_next_mm = [0]
            bg_next_dma = [0]
            LOOKAHEAD = 7

            def bg_prefetch():
                while (
                    bg_next_dma[0] < len(bgq)
                    and bg_next_dma[0] < bg_next_mm[0] + LOOKAHEAD
                ):
                    dma_fn, _ = bgq[bg_next_dma[0]]
                    if dma_fn is not None:
                        dma_fn()
                    bg_next_dma[0] += 1

            def bg_pop(n=1):
                bg_prefetch()
                for _ in range(n):
                    if bg_next_mm[0] >= len(bgq):
                        return
                    _, mm_fn = bgq[bg_next_mm[0]]
                    bg_next_mm[0] += 1
                    mm_fn()
                    bg_prefetch()

            def bg_drain():
                bg_pop(len(bgq) - bg_next_mm[0] + 1)

            # ---- projection tasks (enqueued, popped as bg work)
            def kt_task(v, sl):
                SK = kts[v] * P
                w = min(512, SK - sl * 512)
                x_re = xk_d[v][:, :].rearrange("(a p) k -> p a k", p=P)

                def dma_fn():
                    xs = xtp.tile([P, 8, 512], bf16, name="xs", tag="xt")
                    nc.sync.dma_start(
                        xs[:, :, :w], x_re[:, :, sl * 512:sl * 512 + w]
                    )
                    dma_fn.xs = xs

                def mm_fn():
                    xs = dma_fn.xs
                    pj = psC.tile([P, 512], f32, name="pj", tag="bg")
                    for a in range(8):
                        nc.tensor.matmul(
                            pj[:, :w],
                            lhsT=wk[:, a, :],
                            rhs=xs[:, a, :w],
                            start=(a == 0),
                            stop=(a == 7),
                        )
                    base = off[v] * P + sl * 512
                    nc.vector.tensor_copy(out=KT[:, base:base + w], in_=pj[:, :w])

                return (dma_fn, mm_fn)

            def v_task(v, sl):
                SK = kts[v] * P
                w = min(512, SK - sl * 512)
                nsub = w // P
                x_re = xv_d[v][:, :].rearrange("(a p) k -> p a k", p=P)

                def dma_fn():
                    xs = xtp.tile([P, 8, 512], bf16, name="xvs", tag="xt")
                    nc.sync.dma_start(
                        xs[:, :, :w], x_re[:, :, sl * 512:sl * 512 + w]
                    )
                    dma_fn.xs = xs

                def mm_fn():
                    xs = dma_fn.xs
                    for sub in range(nsub):
                        kt = off[v] + sl * 4 + sub
                        pv = psC.tile([P, 512], f32, name="pv", tag="bg")
                        for a in range(8):
                            nc.tensor.matmul(
                                pv[:, 0:PC],
                                lhsT=xs[:, a, sub * P:(sub + 1) * P],
                                rhs=wv[:, a, :],
                                start=(a == 0),
                                stop=(a == 7),
                            )
                        nc.vector.tensor_copy(
                            out=V[:, kt, :, 0:HD],
                            in_=pv[:, 0:PC].rearrange("p (h c) -> p h c", c=HD),
                        )

                return (dma_fn, mm_fn)

            def qt_task(v, sl):
                x_re = xq_d[v][:, :].rearrange("(a p) q -> p a q", p=P)

                def dma_fn():
                    xs = xtp.tile([P, 8, 512], bf16, name="xqs", tag="xt")
                    nc.sync.dma_start(xs[:], x_re[:, :, sl * 512:(sl + 1) * 512])
                    dma_fn.xs = xs

                def mm_fn():
                    xs = dma_fn.xs
                    pq = psC.tile([P, 512], f32, name="pq", tag="bg")
                    for a in range(8):
                        nc.tensor.matmul(
                            pq[:],
                            lhsT=wq[:, a, :],
                            rhs=xs[:, a, :],
                            start=(a == 0),
                            stop=(a == 7),
                        )
                    nc.vector.tensor_copy(
                        out=QT[:, v, sl * 512:(sl + 1) * 512], in_=pq[:]
                    )

                return (dma_fn, mm_fn)

            def d_task(v, qt):
                # output projection: out_v[qt*128:, :] = OT_v_slice.T @ Wo
                def mm_fn():
                    ob = outp.tile([P, D], bf16, name="ob", tag="ob")
                    for eh in range(2):
                        wd = psC.tile([P, 512], f32, name="wd", tag="bg")
                        nc.tensor.matmul(
                            wd[:],
                            lhsT=OT[:, v, qt * P:(qt + 1) * P],
                            rhs=wo[:, eh * 512:(eh + 1) * 512],
                            start=True,
                            stop=True,
                        )
                        nc.vector.tensor_copy(
                            out=ob[:, eh * 512:(eh + 1) * 512], in_=wd[:]
                        )
                    nc.sync.dma_start(out_d[v][qt * P:(qt + 1) * P, :], ob[:])

                return (None, mm_fn)

            def nslabs(v):
                return (kts[v] * P + 511) // 512

            # D tasks deferred by one chunk so their normalization chain
            # (copy->repack->recip->unpack->bc->mul->shift-DMA) has a chunk's
            # worth of time before a bg pop makes the in-order PE stream
            # wait on it.
            defer_d = []

            # ---- attention for one batch slot
            def attention(v, pops):
                L = kts[v]
                ko = off[v]
                for ch in range(4):  # 512-wide q chunks
                    bgq.extend(defer_d)
                    defer_d.clear()
                    q0 = ch * 512
                    av = psB.tile([HD + 1, 1024], f32, name="av", tag="av")
                    for kt in range(L):
                        bg_pop(pops)
                        sc = psA.tile([P, 1024], f32, name="sc", tag="sc")
                        for s in range(2):
                            pb = s * HD
                            nc.tensor.matmul(
                                sc[:, s * 512:(s + 1) * 512],
                                lhsT=KT[
                                    pb:pb + HD,
                                    (ko + kt) * P:(ko + kt + 1) * P,
                                ],
                                rhs=QT[pb:pb + HD, v, q0:q0 + 512],
                                start=True,
                                stop=True,
                            )
                        ex = expp.tile([P, 1024], bf16, name="ex", tag="ex")
                        nc.scalar.activation(
                            ex[:],
                            sc[:],
                            AF.Exp,
                            bias=mb[:, ko + kt:ko + kt + 1],
                            scale=msc[:, ko + kt:ko + kt + 1],
                        )
                        for s in range(2):
                            nc.tensor.matmul(
                                av[:, s * 512:(s + 1) * 512],
                                lhsT=V[:, ko + kt, s, :],
                                rhs=ex[:, s * 512:(s + 1) * 512],
                                start=(kt == 0),
                                stop=(kt == L - 1),
                            )
                    # normalization.  A [1,1024] reciprocal on the DVE costs
                    # ~6.4ns per free-dim column, so repack the denominator
                    # row to [8,128] (via a DRAM bounce — DMA is the only
                    # engine that can move data across partitions) and run
                    # the reciprocal over 128 columns instead of 1024.
                    avb = nrm.tile([HD + 1, 1024], bf16, name="avb", tag="rcp")
                    nc.vector.tensor_copy(out=avb[:], in_=av[:])
                    dct = nrm.tile([8, P], bf16, name="dct", tag="dc")
                    nc.sync.dma_start(
                        dct[0:8, :],
                        avb[HD:HD + 1, :].rearrange("o (p c) -> o p c", p=8),
                    )
                    with nc.allow_low_precision(
                        reason="softmax denominators are O(1e3); bf16 "
                        "reciprocal keeps enough digits for attention"
                    ):
                        nc.vector.reciprocal(dct[0:8, :], dct[0:8, :])
                    rrb = nrm.tile([1, 1024], bf16, name="rrb", tag="rrb")
                    nc.sync.dma_start(
                        rrb[:].rearrange("o (p c) -> o p c", p=8),
                        dct[0:8, :],
                    )
                    for s in range(2):
                        bc = psC.tile([P, 512], f32, name="bc", tag="bg")
                        nc.tensor.matmul(
                            bc[:],
                            lhsT=ones1[:, :],
                            rhs=rrb[:, s * 512:(s + 1) * 512],
                            start=True,
                            stop=True,
                        )
                        if s == 0:
                            nc.vector.tensor_mul(
                                out=OT[0:HD, v, q0:q0 + 512],
                                in0=avb[0:HD, 0:512],
                                in1=bc[0:HD, :],
                            )
                        else:
                            # DVE lanes can't shift partitions; write the odd
                            # head at partitions 0:64 and DMA it to 64:128
                            ot1 = nrm.tile([HD, 512], bf16, name="ot1", tag="ot1")
                            nc.vector.tensor_mul(
                                out=ot1[:],
                                in0=avb[0:HD, 512:1024],
                                in1=bc[0:HD, :],
                            )
                            nc.sync.dma_start(
                                OT[HD:P, v, q0:q0 + 512], ot1[:]
                            )
                    for qt in range(ch * 4, ch * 4 + 4):
                        defer_d.append(d_task(v, qt))

            # ---- emission schedule.
            # Only slab 0 of KT/V/QT(v0) is emitted inline (the PE stream is
            # in-order: anything emitted before the first scores MM would
            # make it transitively wait on that DMA).  Everything else goes
            # through the FIFO, popped one group per attention k-tile.
            kt0, v0, qt0 = kt_task(0, 0), v_task(0, 0), qt_task(0, 0)
            for dma_fn, _ in (kt0, v0, qt0):
                dma_fn()
            for _, mm_fn in (kt0, v0, qt0):
                mm_fn()
            for sl in range(1, nslabs(0)):
                bgq.append(kt_task(0, sl))
                bgq.append(v_task(0, sl))
            for sl in range(1, 4):
                bgq.append(qt_task(0, sl))
            for vv in range(1, NB):
                for sl in range(nslabs(vv)):
                    bgq.append(kt_task(vv, sl))
                    bgq.append(v_task(vv, sl))
                for sl in range(4):
                    bgq.append(qt_task(vv, sl))

            for v in range(NB):
                attention(v, pops=1 if v == 0 else 2)
            bgq.extend(defer_d)
            defer_d.clear()
            bg_drain()

    _split_multi_waits(nc, mybir)
    return nc


def _get_program(kts: tuple):
    if kts not in _PROG_CACHE:
        _PROG_CACHE[kts] = _build_program(kts)
    return _PROG_CACHE[kts]


def kernel(**inputs) -> np.ndarray:
    import ml_dtypes
    from concourse.bass_utils import run_bass_kernel_spmd

    q = np.asarray(inputs["queries"], dtype=np.float32)
    k = np.asarray(inputs["keys"], dtype=np.float32)
    v = np.asarray(inputs["values"], dtype=np.float32)
    vl = np.asarray(inputs["valid_lens"]).astype(np.int64)
    Wq = np.asarray(inputs["Wq"], dtype=np.float32)
    Wk = np.asarray(inputs["Wk"], dtype=np.float32)
    Wv = np.asarray(inputs["Wv"], dtype=np.float32)
    Wo = np.asarray(inputs["Wo"], dtype=np.float32)

    bf = ml_dtypes.bfloat16
    nkt = np.array(
        [S // P if vl[b] == 0 else min(S // P, int(math.ceil(vl[b] / P)))
         for b in range(B)],
        dtype=np.int64,
    )
    # Slot order: largest batch FIRST (its long attention span hides the
    # other batches' projection DMAs and matmuls), second-largest LAST (its
    # span hides the smaller batches' normalization chains and output
    # projections, leaving only its own last chunk's chain as tail), the
    # small batches in between.
    srt = np.argsort(nkt, kind="stable")
    order = np.array([srt[-1], *srt[0:-2], srt[-2]])
    kts = tuple(int(nkt[b]) for b in order)
    nc = _get_program(kts)

    # masks are identical across cores: [128, SL] per-(k-partition, k-tile)
    mbs, mss = [], []
    for vv, b in enumerate(order):
        L = kts[vv]
        kk = (np.arange(L)[None, :] * P + np.arange(P)[:, None]).astype(np.int64)
        vlb = int(vl[b])
        if vlb == 0:
            mbs.append(np.zeros((P, L), np.float32))
            mss.append(np.zeros((P, L), np.float32))
        else:
            mbs.append(np.where(kk < vlb, 0.0, NEG).astype(np.float32))
            mss.append(np.full((P, L), 1.0 / math.sqrt(HD), np.float32))
    m_bias = np.concatenate(mbs, axis=1)
    m_scale = np.concatenate(mss, axis=1)

    xqs = [np.ascontiguousarray(q[b].T).astype(bf) for b in order]
    xks = [np.ascontiguousarray(k[b].T[:, :kts[vv] * P]).astype(bf)
           for vv, b in enumerate(order)]
    xvs = [np.ascontiguousarray(v[b].T[:, :kts[vv] * P]).astype(bf)
           for vv, b in enumerate(order)]

    in_maps = []
    for c in range(NCORES):
        cols = slice(c * PC, (c + 1) * PC)
        im = {
            "wq": np.ascontiguousarray(Wq[:, cols]).astype(bf),
            "wk": np.ascontiguousarray(Wk[:, cols]).astype(bf),
            "wv": np.ascontiguousarray(Wv[:, cols]).astype(bf),
            "wo": np.ascontiguousarray(Wo[cols, :]).astype(bf),
            "mb": m_bias,
            "ms": m_scale,
        }
        for vv in range(len(kts)):
            im[f"xq{vv}"] = xqs[vv]
            im[f"xk{vv}"] = xks[vv]
            im[f"xv{vv}"] = xvs[vv]
        in_maps.append(im)

    globals()["_LAST_IN_MAPS"] = in_maps
    res = run_bass_kernel_spmd(nc, in_maps, list(range(NCORES))).results

    out = np.zeros((B, S, D), dtype=np.float32)
    for vv, b in enumerate(order):
        for c in range(NCORES):
            out[b] += res[c][f"out{vv}"].astype(np.float32)
    return out
